# revision 1
# baseline (speedup 1.0000x reference)
"""Trainium2 Bass kernel for -mean(antonymy_score > synonymy_score).

Strategy: pure data-parallel over 8 NeuronCores. Each core receives a
contiguous 1/8 slice of the batch. On the host, antonymy/synonymy are
interleaved into one flat "pairs" tensor laid out as consecutive
[128, 2, fd_j] chunk blocks so each chunk arrives in a single contiguous
DMA carrying one semaphore (the walrus codegen path here allows only one
sync wait per instruction). Chunks alternate between the two HWDGE rings
(SP + ACT); small leading chunks unblock the second ring quickly (the
shared descriptor-gen hardware serializes the first DMA per ring, ~3us),
and tapered trailing chunks minimize the compute left after the stream
drains. The stream runs at the ~358 GB/s per-core HBM bandwidth limit.

Each chunk is consumed by one fused DVE scalar_tensor_tensor:
mask = (ant bypass 0) is_gt syn, with accum_out producing the
per-partition free-dim sum in the same pass. The DVE is gated on a
mid-stream chunk semaphore so compute starts late and finishes right as
the stream ends. Partial counts [128, n_chunks] are DMA'd back and the
host computes -total/B (exact: integer-valued fp32 counts). S1_out is
unused by the computation - it only fixes the batch size.

Raw Bass (no TileContext); one semaphore per chunk DMA (two in-flight
DMAs sharing a sem can interleave their 16 per-SDMA-engine increments,
so cumulative waits would race). The framework-emitted const-tensor
memsets and the entry/exit all-engine barriers are stripped: this
program's data flow is fully ordered by explicit semaphores, and the
barriers only delay DMA start / the runtime teardown.
"""

from contextlib import ExitStack

import numpy as np

import concourse.bass as bass
import concourse.mybir as mybir
from concourse.bass_utils import run_bass_kernel_spmd

B = 8388608
N_CORES = 8
PER_CORE = B // N_CORES  # 1048576
P = 128
FD_TOTAL = PER_CORE // P  # 8192 per array per core

# Per-chunk free-dim sizes (per array). Even indices go to the SP ring,
# odd to the ACT ring.
CHUNK_FDS = [256, 256, 2048, 2048, 384, 1024, 1024, 512, 384, 256]
GATE_IDX = 4  # DVE starts once this chunk has landed
assert sum(CHUNK_FDS) == FD_TOTAL
N_CHUNKS = len(CHUNK_FDS)
CHUNK_OFFS = np.concatenate([[0], np.cumsum(CHUNK_FDS)]).tolist()

F32 = mybir.dt.float32

_NC = None


def build_nc():
    nc = bass.Bass()
    pairs = nc.dram_tensor("pairs", [2 * PER_CORE], F32, kind="ExternalInput")
    out = nc.dram_tensor("out", [P, N_CHUNKS], F32, kind="ExternalOutput")

    with ExitStack() as ctx:
        pair_buf = ctx.enter_context(
            nc.sbuf_tensor("pair_buf", [P, 2 * FD_TOTAL], F32)
        )
        mask_buf = ctx.enter_context(nc.sbuf_tensor("mask_buf", [P, FD_TOTAL], F32))
        partials = ctx.enter_context(nc.sbuf_tensor("partials", [P, N_CHUNKS], F32))
        chunk_sems = [
            ctx.enter_context(nc.semaphore(f"chunk{k}")) for k in range(N_CHUNKS)
        ]
        dve_sem = ctx.enter_context(nc.semaphore("dve_sem"))
        out_sem = ctx.enter_context(nc.semaphore("out_sem"))
        block = ctx.enter_context(nc.Block())

        def chunk_dma(eng, k):
            fd = CHUNK_FDS[k]
            off = CHUNK_OFFS[k]
            src = bass.AP(pairs, 2 * P * off, [[2 * fd, P], [1, 2 * fd]])
            dst = pair_buf[:, 2 * off : 2 * (off + fd)]
            eng.dma_start(dst, src).then_inc(chunk_sems[k], 16)

        @block.sync
        def _(sync: bass.BassEngine):
            for k in range(0, N_CHUNKS, 2):
                chunk_dma(sync, k)
            sync.wait_ge(dve_sem, N_CHUNKS)
            sync.dma_start(out[:], partials[:]).then_inc(out_sem, 16)

        @block.scalar
        def _(scalar: bass.BassEngine):
            for k in range(1, N_CHUNKS, 2):
                chunk_dma(scalar, k)

        @block.vector
        def _(vector: bass.BassEngine):
            vector.wait_ge(chunk_sems[GATE_IDX], 16)
            for k in range(N_CHUNKS):
                fd = CHUNK_FDS[k]
                off = CHUNK_OFFS[k]
                vector.wait_ge(chunk_sems[k], 16)
                # mask = (ant bypass 0.0) is_gt syn -> 1.0/0.0
                # partials[:, k] = free-dim sum of mask (same instruction)
                vector.scalar_tensor_tensor(
                    out=mask_buf[:, off : off + fd],
                    in0=pair_buf[:, 2 * off : 2 * off + fd],
                    scalar=0.0,
                    in1=pair_buf[:, 2 * off + fd : 2 * (off + fd)],
                    op0=mybir.AluOpType.bypass,
                    op1=mybir.AluOpType.is_gt,
                    accum_out=partials[:, k : k + 1],
                ).then_inc(dve_sem, 1)

    _strip_framework_barriers(nc)
    return nc


def _strip_framework_barriers(nc):
    """Bass.__init__ unconditionally materializes four const SBUF tensors
    (gpsimd memsets) plus an all-engine barrier before main, and Block
    exit emits another all-engine barrier. This kernel reads none of the
    consts, and its data flow is fully ordered by explicit semaphores,
    so drop all of it: the entry barrier delays the first DMA (and the
    memsets would open the profiled exec window early); the exit barrier
    makes every engine wait for the last one before starting the
    runtime's teardown ladder."""
    for bb in nc.main_func.blocks:
        if bb.name != "main" and not bb.name.endswith("_end"):
            continue

        def removable(ins):
            t = type(ins).__name__
            if t == "InstMemset":
                return getattr(ins.outs[0], "memref", "").startswith("const-")
            return t in ("InstDrain", "InstEventSemaphore")

        bb.instructions[:] = [
            ins for ins in bb.instructions if not removable(ins)
        ]


def _make_pairs(synonymy_score, antonymy_score):
    """Build the per-core flat pair tensor: consecutive [128, 2, fd_j]
    blocks with ant rows first (in0), then syn rows (in1)."""
    syn = np.asarray(synonymy_score, dtype=np.float32).reshape(
        N_CORES, P, FD_TOTAL
    )
    ant = np.asarray(antonymy_score, dtype=np.float32).reshape(
        N_CORES, P, FD_TOTAL
    )
    blocks = []
    for k in range(N_CHUNKS):
        s, e = CHUNK_OFFS[k], CHUNK_OFFS[k + 1]
        blk = np.stack([ant[:, :, s:e], syn[:, :, s:e]], axis=2)  # [C,P,2,fd]
        blocks.append(blk.reshape(N_CORES, -1))
    return np.ascontiguousarray(np.concatenate(blocks, axis=1))  # [C, 2*PER_CORE]


def run(inputs, trace=False, trace_cores=None):
    """Run the SPMD kernel on 8 cores. Returns (result_scalar, BassKernelResults)."""
    global _NC
    if _NC is None:
        _NC = build_nc()

    pairs = _make_pairs(inputs["synonymy_score"], inputs["antonymy_score"])
    in_maps = [{"pairs": pairs[c]} for c in range(N_CORES)]
    try:
        bkr = run_bass_kernel_spmd(
            _NC,
            in_maps,
            list(range(N_CORES)),
            trace=trace,
            trace_cores=trace_cores,
        )
    except Exception:
        # A crashed prior process can leave the accelerator in a transient
        # "unrecoverable" state that clears on the next attempt.
        bkr = run_bass_kernel_spmd(
            _NC,
            in_maps,
            list(range(N_CORES)),
            trace=trace,
            trace_cores=trace_cores,
        )
    total = sum(
        np.asarray(r["out"], dtype=np.float64).sum() for r in bkr.results
    )
    result = np.float32(-(total / B))
    return result, bkr


def kernel(S1_out, synonymy_score, antonymy_score):
    result, _ = run(
        {"synonymy_score": synonymy_score, "antonymy_score": antonymy_score}
    )
    return result

